# revision 11
# baseline (speedup 1.0000x reference)
"""Trainium2 Bass kernel for the segment_reduce problem (nn_Network_21010980012235).

Math: the reference's mean-centering cancels out:
    out[m] = [(coords[pid[m]] - min_c[cid[m]]) * s[cid[m]], feats[pid[m]]]
with per-cluster k:  s = min(fullscale / max_c(max_c - min_c) - 0.01, scale)
computed from raw gathered coords min/max only.

Sharding: 256 clusters -> 8 cores x 32 clusters. Host pads each cluster to S
rows (multiple of 128) by duplicating one of the cluster's own rows (doesn't
change min/max; padded output rows are dropped on the host).

The host concatenates [coords | feats] into one [N, 35] table so a single
indirect-DMA stream gathers whole 140 B output rows. TRN2's indirect DMA
moves one index per partition per instruction (block semantics), so each
cluster's S rows take S/128 gather instructions of [128, 35].

Per cluster (fully pipelined, no cross-cluster barrier):
  gather -> strided DVE min/max reduce -> tiny PE-transpose param chain ->
  K=1 matmul partition-broadcast -> in-place affine transform of the coords
  columns -> one contiguous store of the [128, G*35] tile.
"""

import sys

sys.path.insert(0, "/opt/trn_rl_repo")

import numpy as np

P = 128
C = 32
D = 3 + C  # 35 floats per combined row
K = 256
NCORES = 8
KPC = K // NCORES  # clusters per core

_CACHE = {}


def _build(N, S, fullscale, scale, kpc=KPC, repeat=1):
    import concourse.bass as bass
    from concourse import bacc, mybir, tile
    from concourse.masks import make_identity

    G = S // P
    assert S % P == 0
    f32 = mybir.dt.float32
    Alu = mybir.AluOpType
    X = mybir.AxisListType.X

    nc = bacc.Bacc("TRN2", target_bir_lowering=False, debug=False)
    pidx = nc.dram_tensor("pidx", [kpc, S], mybir.dt.int32, kind="ExternalInput")
    comb = nc.dram_tensor("comb", [N, D], f32, kind="ExternalInput")
    outt = nc.dram_tensor("outt", [kpc * S, D], f32, kind="ExternalOutput")

    # gather chunk g pulls cluster rows {p*G+g}: partition p holds rows
    # [p*G, (p+1)*G) across its G slots -> contiguous per-partition store
    pidx_v = pidx.ap().rearrange("j (p g) -> j p g", p=P)
    out_v = outt.ap().rearrange("(j p g) c -> j p (g c)", j=kpc, p=P)

    with tile.TileContext(nc) as tc:
        with tc.tile_pool(name="idxp", bufs=4) as idxp, tc.tile_pool(
            name="fp", bufs=4
        ) as fp, tc.tile_pool(name="smp", bufs=3) as smp, tc.tile_pool(
            name="cst", bufs=1
        ) as cst, tc.tile_pool(name="pp", bufs=1, space="PSUM") as pp:
            ident = cst.tile([P, P], f32, tag="ident")
            make_identity(nc, ident[:])
            ones = cst.tile([1, P], f32, tag="ones")
            nc.vector.memset(ones[:], 1.0)

            for _r in range(repeat):
                for j in range(kpc):
                    idxt = idxp.tile([P, G], mybir.dt.int32, tag="idx")
                    nc.sync.dma_start(idxt[:], pidx_v[j])
                    ft = fp.tile([P, G * D], f32, tag="ft")
                    for g in range(G):
                        nc.gpsimd.indirect_dma_start(
                            out=ft[:, g * D : (g + 1) * D],
                            out_offset=None,
                            in_=comb.ap(),
                            in_offset=bass.IndirectOffsetOnAxis(
                                ap=idxt[:, g : g + 1], axis=0
                            ),
                        )
                    # strided view of the coords columns: [128, 3, G]
                    cg = ft[:].rearrange("p (g c) -> p c g", c=D)[:, 0:3, :]
                    mn = smp.tile([P, 3], f32, tag="mn")
                    mx = smp.tile([P, 3], f32, tag="mx")
                    nc.vector.tensor_reduce(out=mn[:], in_=cg, axis=X, op=Alu.min)
                    nc.vector.tensor_reduce(out=mx[:], in_=cg, axis=X, op=Alu.max)
                    # cross-partition: transpose [128,3] -> [3,128], reduce
                    tpmn = pp.tile([3, P], f32, tag="tpmn")
                    tpmx = pp.tile([3, P], f32, tag="tpmx")
                    nc.tensor.transpose(tpmn[:], mn[:], ident[:])
                    nc.tensor.transpose(tpmx[:], mx[:], ident[:])
                    mn3 = smp.tile([3, 1], f32, tag="mn3")
                    rg3 = smp.tile([3, 1], f32, tag="rg3")
                    nc.vector.tensor_reduce(out=mn3[:], in_=tpmn[:], axis=X, op=Alu.min)
                    nc.vector.tensor_reduce(out=rg3[:], in_=tpmx[:], axis=X, op=Alu.max)
                    nc.vector.tensor_tensor(
                        out=rg3[:], in0=rg3[:], in1=mn3[:], op=Alu.subtract
                    )
                    # rows [1,3] on partition 0
                    trmn = pp.tile([1, 3], f32, tag="trmn")
                    trrg = pp.tile([1, 3], f32, tag="trrg")
                    nc.tensor.transpose(trmn[:], mn3[:], ident[:3, :3])
                    nc.tensor.transpose(trrg[:], rg3[:], ident[:3, :3])
                    prow = smp.tile([1, 4], f32, tag="prow")
                    nc.vector.tensor_copy(out=prow[:, 0:3], in_=trmn[:])
                    rmax = smp.tile([1, 1], f32, tag="rmax")
                    nc.vector.tensor_reduce(out=rmax[:], in_=trrg[:], axis=X, op=Alu.max)
                    recip = smp.tile([1, 1], f32, tag="recip")
                    nc.vector.reciprocal(recip[:], rmax[:])
                    nc.vector.tensor_scalar(
                        out=prow[:, 3:4],
                        in0=recip[:],
                        scalar1=float(fullscale),
                        scalar2=0.01,
                        op0=Alu.mult,
                        op1=Alu.subtract,
                    )
                    nc.vector.tensor_scalar(
                        out=prow[:, 3:4],
                        in0=prow[:, 3:4],
                        scalar1=float(scale),
                        scalar2=None,
                        op0=Alu.min,
                    )
                    # broadcast params to all partitions via K=1 matmul
                    ppar = pp.tile([P, 4], f32, tag="ppar")
                    nc.tensor.matmul(
                        ppar[:], lhsT=ones[:], rhs=prow[:], start=True, stop=True
                    )
                    pj = smp.tile([P, 4], f32, tag="pj")
                    nc.vector.tensor_copy(out=pj[:], in_=ppar[:])
                    # in-place affine transform of coords columns
                    v3 = ft[:].rearrange("p (g c) -> p g c", c=D)[:, :, 0:3]
                    bmin = pj[:, 0:3].unsqueeze(1).to_broadcast([P, G, 3])
                    nc.vector.tensor_tensor(out=v3, in0=v3, in1=bmin, op=Alu.subtract)
                    nc.vector.tensor_scalar(
                        out=v3,
                        in0=v3,
                        scalar1=pj[:, 3:4],
                        scalar2=None,
                        op0=Alu.mult,
                    )
                    nc.sync.dma_start(out_v[j], ft[:])

    nc.compile()
    return nc


def _get_runner(N, S, fullscale, scale, n_cores=NCORES, repeat=1):
    """Compile (once) and return (fn, meta)."""
    key = (N, S, float(fullscale), float(scale), n_cores, repeat)
    if key in _CACHE:
        return _CACHE[key]

    import jax
    from jax.sharding import Mesh, NamedSharding, PartitionSpec

    try:
        from jax.experimental.shard_map import shard_map
    except ImportError:
        from jax.shard_map import shard_map

    from concourse import bass2jax, mybir

    nc = _build(N, S, fullscale, scale, repeat=repeat)
    bass2jax.install_neuronx_cc_hook()

    partition_name = nc.partition_id_tensor.name if nc.partition_id_tensor else None
    in_names = []
    out_names = []
    out_avals = []
    for alloc in nc.m.functions[0].allocations:
        if not isinstance(alloc, mybir.MemoryLocationSet):
            continue
        name = alloc.memorylocations[0].name
        if alloc.kind == "ExternalInput":
            if name != partition_name:
                in_names.append(name)
        elif alloc.kind == "ExternalOutput":
            out_names.append(name)
            out_avals.append(
                jax.core.ShapedArray(
                    tuple(alloc.tensor_shape), mybir.dt.np(alloc.dtype)
                )
            )
    n_params = len(in_names)
    all_in_names = in_names + out_names
    if partition_name is not None:
        all_in_names = all_in_names + [partition_name]

    def _body(*args):
        operands = list(args)
        if partition_name is not None:
            operands.append(bass2jax.partition_id_tensor())
        outs = bass2jax._bass_exec_p.bind(
            *operands,
            out_avals=tuple(out_avals),
            in_names=tuple(all_in_names),
            out_names=tuple(out_names),
            lowering_input_output_aliases=(),
            sim_require_finite=True,
            sim_require_nnan=True,
            nc=nc,
        )
        return tuple(outs)

    devices = jax.devices()[:n_cores]
    mesh = Mesh(np.asarray(devices), ("core",))
    n_outs = len(out_names)
    sharded = jax.jit(
        shard_map(
            _body,
            mesh=mesh,
            in_specs=(PartitionSpec("core"),) * (n_params + n_outs),
            out_specs=(PartitionSpec("core"),) * n_outs,
            check_rep=False,
        ),
        donate_argnums=tuple(range(n_params, n_params + n_outs)),
        keep_unused=True,
    )
    sh = NamedSharding(mesh, PartitionSpec("core"))
    zeros_fn = jax.jit(
        lambda: tuple(
            jax.numpy.zeros((n_cores * a.shape[0], *a.shape[1:]), a.dtype)
            for a in out_avals
        ),
        out_shardings=(sh,) * n_outs,
    )
    meta = dict(
        in_names=in_names,
        out_names=out_names,
        out_avals=out_avals,
        nc=nc,
        mesh=mesh,
        zeros_fn=zeros_fn,
    )
    _CACHE[key] = (sharded, meta)
    return sharded, meta


def _prep(clusters_idx, feats, coords):
    cid = clusters_idx[:, 0].astype(np.int64)
    pid = np.ascontiguousarray(clusters_idx[:, 1]).astype(np.int32)
    M = cid.shape[0]
    starts = np.searchsorted(cid, np.arange(K), side="left")
    ends = np.searchsorted(cid, np.arange(K), side="right")
    sizes = ends - starts
    S = max((int(sizes.max()) + P - 1) // P * P, P)
    first = pid[np.minimum(starts, M - 1)].astype(np.int32)
    pidx = np.broadcast_to(first[:, None], (K, S)).copy()
    ranks = np.arange(M, dtype=np.int64) - starts[cid]
    pidx[cid, ranks] = pid
    comb = np.empty((feats.shape[0], D), np.float32)
    comb[:, 0:3] = coords
    comb[:, 3:] = feats
    g = cid * S + ranks
    return pidx, comb, S, g


def kernel(clusters_idx, feats, coords, fullscale, scale):
    clusters_idx = np.asarray(clusters_idx)
    feats = np.ascontiguousarray(np.asarray(feats), dtype=np.float32)
    coords = np.ascontiguousarray(np.asarray(coords), dtype=np.float32)
    N = feats.shape[0]

    pidx, comb, S, g = _prep(clusters_idx, feats, coords)
    sharded, meta = _get_runner(N, S, fullscale, scale)

    percore = {
        "pidx": [pidx[d * KPC : (d + 1) * KPC] for d in range(NCORES)],
        "comb": [comb] * NCORES,
    }
    concat_in = [np.concatenate(percore[name], axis=0) for name in meta["in_names"]]
    out_arrs = sharded(*concat_in, *meta["zeros_fn"]())
    outs = {name: np.asarray(out_arrs[i]) for i, name in enumerate(meta["out_names"])}
    out_full = outs["outt"].reshape(K * S, D)
    return out_full[g]
